# revision 11
# baseline (speedup 1.0000x reference)
import math
from contextlib import ExitStack

import ml_dtypes
import numpy as np

import concourse.bacc as bacc
import concourse.bass as bass
import concourse.tile as tile
from concourse import mybir
from concourse.alu_op_type import AluOpType
from concourse.bass_utils import run_bass_kernel_spmd
from concourse.masks import make_identity

B, NC, ND = 64, 1024, 1024
DX, DY, DH = 64, 64, 512
NH, HD = 8, 64
CH = 4
NCORES = 8
TPC = B // NCORES

F32 = mybir.dt.float32
F32R = mybir.dt.float32r
BF16 = mybir.dt.bfloat16
AF = mybir.ActivationFunctionType

SCALE = 1.0 / math.sqrt(DH)
MM_DT = F32R


def _blocks(n, step=512):
    return [(s, min(s + step, n)) for s in range(0, n, step)]


def build_nc(n_tasks, ncp):
    KCP = ncp // 128
    MS = _blocks(ncp)
    QS = _blocks(ND)

    nc = bacc.Bacc(None, target_bir_lowering=False, dynamic_dma_scratch_size=256)

    def r(ap):
        return ap.bitcast(MM_DT)

    def mm(out, lhsT, rhs, **kw):
        nc.tensor.matmul(out, lhsT, rhs, **kw)

    CT_d = nc.dram_tensor("CT", [n_tasks, DX + DY, ncp], F32, kind="ExternalInput")
    XCT_d = nc.dram_tensor("XCT", [n_tasks, DX, ncp], F32, kind="ExternalInput")
    XDT_d = nc.dram_tensor("XDT", [n_tasks, DX, ND], F32, kind="ExternalInput")
    VAL_d = nc.dram_tensor("VALID", [n_tasks, ncp], F32, kind="ExternalInput")
    W0_d = nc.dram_tensor("W0", [DX + DY, DH], F32, kind="ExternalInput")
    b0_d = nc.dram_tensor("b0", [DH], F32, kind="ExternalInput")
    WmB_d = nc.dram_tensor("WmB", [2, DH, DH], BF16, kind="ExternalInput")
    bm_d = nc.dram_tensor("bm", [2, DH], F32, kind="ExternalInput")
    WlB_d = nc.dram_tensor("WlB", [DH, DH], BF16, kind="ExternalInput")
    bl_d = nc.dram_tensor("bl", [DH], F32, kind="ExternalInput")
    temb_d = nc.dram_tensor("task_emb", [DH], F32, kind="ExternalInput")
    Wq0_d = nc.dram_tensor("Wq0", [DX, DH], F32, kind="ExternalInput")
    Wk0_d = nc.dram_tensor("Wk0", [DX, DH], F32, kind="ExternalInput")
    Wv0B_d = nc.dram_tensor("Wv0B", [DH, DH], BF16, kind="ExternalInput")
    Wo0_d = nc.dram_tensor("Wo0", [DH, DH], F32, kind="ExternalInput")
    bo0_d = nc.dram_tensor("bo0", [DH], F32, kind="ExternalInput")
    Wq1_d = nc.dram_tensor("Wq1", [DH, DH], F32, kind="ExternalInput")
    Wk1_d = nc.dram_tensor("Wk1", [DX, DH], F32, kind="ExternalInput")
    Wv1B_d = nc.dram_tensor("Wv1B", [DH, DH], BF16, kind="ExternalInput")
    Wo1_d = nc.dram_tensor("Wo1", [DH, DH], F32, kind="ExternalInput")
    bo1_d = nc.dram_tensor("bo1", [DH], F32, kind="ExternalInput")
    OUT_d = nc.dram_tensor("OUT", [n_tasks, ND, DH], F32, kind="ExternalOutput")

    with tile.TileContext(nc) as tc, ExitStack() as ctx, \
            nc.allow_low_precision(reason="fp32r/bf16 rounding is intentional"):
        wp = ctx.enter_context(tc.tile_pool(name="wp", bufs=1))
        czp = ctx.enter_context(tc.tile_pool(name="czp", bufs=2))
        xp = ctx.enter_context(tc.tile_pool(name="xp", bufs=3))
        maskp = ctx.enter_context(tc.tile_pool(name="maskp", bufs=2))
        hp = ctx.enter_context(tc.tile_pool(name="hp", bufs=2))
        dp = ctx.enter_context(tc.tile_pool(name="dp", bufs=2))
        vep = ctx.enter_context(tc.tile_pool(name="vep", bufs=12))
        qp = ctx.enter_context(tc.tile_pool(name="qp", bufs=3))
        kp = ctx.enter_context(tc.tile_pool(name="kp", bufs=2))
        eeop = ctx.enter_context(tc.tile_pool(name="eeop", bufs=3))
        ftp = ctx.enter_context(tc.tile_pool(name="ftp", bufs=2))
        bcp = ctx.enter_context(tc.tile_pool(name="bcp", bufs=2))
        stp = ctx.enter_context(tc.tile_pool(name="stp", bufs=2))
        urp = ctx.enter_context(tc.tile_pool(name="urp", bufs=2))
        accp = ctx.enter_context(tc.tile_pool(name="accp", bufs=2, space="PSUM"))
        orwp = ctx.enter_context(tc.tile_pool(name="orwp", bufs=2, space="PSUM"))
        lgp = ctx.enter_context(tc.tile_pool(name="lgp", bufs=2, space="PSUM"))

        def psum():
            return accp.tile([128, 512], F32, name="ps", tag="ps")

        ident = wp.tile([128, 128], F32, name="ident")
        make_identity(nc, ident)
        ones1f = wp.tile([1, 128], F32, name="ones1f")
        nc.vector.memset(ones1f, 1.0)
        ones1 = wp.tile([1, 128], F32, name="ones1")
        nc.vector.tensor_copy(r(ones1), ones1f)
        ones8 = wp.tile([128, NH], BF16, name="ones8")
        nc.vector.memset(ones8, 1.0)
        d2 = wp.tile([1, 128], mybir.dt.int32, name="d2")
        nc.gpsimd.iota(d2, pattern=[[1, 2], [0, 64]], channel_multiplier=0)
        d2f = wp.tile([1, 128], F32, name="d2f")
        nc.vector.tensor_copy(d2f, d2)
        selE = wp.tile([1, 128], F32, name="selE")
        nc.vector.tensor_scalar(out=r(selE), in0=d2f, scalar1=0.0, scalar2=None,
                                op0=AluOpType.is_equal)
        selO = wp.tile([1, 128], F32, name="selO")
        nc.vector.tensor_scalar(out=r(selO), in0=d2f, scalar1=1.0, scalar2=None,
                                op0=AluOpType.is_equal)

        def load_w_f32r(name, src_ap):
            t = wp.tile([128, CH, DH], F32, name=name, tag=name)
            nc.sync.dma_start(
                out=r(t), in_=src_ap.rearrange("(c p) n -> p c n", p=128).bitcast(MM_DT))
            return t

        def load_w_bf16(name, src_ap):
            t = wp.tile([128, CH, DH], BF16, name=name, tag=name)
            nc.sync.dma_start(out=t, in_=src_ap.rearrange("(c p) n -> p c n", p=128))
            return t

        def load_col(name, src_ap):
            t = wp.tile([128, CH], F32, name=name, tag=name)
            nc.sync.dma_start(out=t, in_=src_ap.rearrange("(c p) -> p c", p=128))
            return t

        W0s = wp.tile([128, DH], F32, name="W0s")
        nc.sync.dma_start(out=r(W0s), in_=W0_d[:, :].bitcast(MM_DT))
        Wm0s = load_w_bf16("Wm0s", WmB_d[0])
        Wm1s = load_w_bf16("Wm1s", WmB_d[1])
        Wls = load_w_bf16("Wls", WlB_d[:, :])
        Wv0s = load_w_bf16("Wv0s", Wv0B_d[:, :])
        Wv1s = load_w_bf16("Wv1s", Wv1B_d[:, :])
        Wo0s = load_w_f32r("Wo0s", Wo0_d[:, :])
        Wq1s = load_w_f32r("Wq1s", Wq1_d[:, :])
        Wo1s = load_w_f32r("Wo1s", Wo1_d[:, :])

        Wq0s = wp.tile([64, DH], F32, name="Wq0s")
        nc.sync.dma_start(out=r(Wq0s), in_=Wq0_d[:, :].bitcast(MM_DT))
        Wk0s = wp.tile([64, DH], F32, name="Wk0s")
        nc.sync.dma_start(out=r(Wk0s), in_=Wk0_d[:, :].bitcast(MM_DT))
        Wk1s = wp.tile([64, DH], F32, name="Wk1s")
        nc.sync.dma_start(out=r(Wk1s), in_=Wk1_d[:, :].bitcast(MM_DT))

        b0c = load_col("b0c", b0_d[:])
        bm0c = load_col("bm0c", bm_d[0])
        bm1c = load_col("bm1c", bm_d[1])
        blc = load_col("blc", bl_d[:])
        tembc = load_col("tembc", temb_d[:])
        bo0c = load_col("bo0c", bo0_d[:])
        dbc = wp.tile([128, CH], F32, name="dbc")
        nc.vector.tensor_add(dbc, blc, tembc)
        bo1r = wp.tile([1, DH], F32, name="bo1r")
        nc.sync.dma_start(out=r(bo1r),
                          in_=bo1_d[:].rearrange("(a n) -> a n", a=1).bitcast(MM_DT))

        def proj64(Ws, xT, outT, width_blocks):
            for c in range(CH):
                for (s, e) in width_blocks:
                    w = e - s
                    ps = psum()
                    mm(ps[:, 0:w], r(Ws[:, c * 128:(c + 1) * 128]), r(xT[:, s:e]),
                       start=True, stop=True)
                    nc.vector.tensor_copy(r(outT[:, c, s:e]), ps[:, 0:w])

        def attn_block(qT, kT, ve_list):
            for hp2 in range(NH // 2):
                he, ho, c = 2 * hp2, 2 * hp2 + 1, hp2
                for (qs0, qs1) in QS:
                    qs = slice(qs0, qs1)
                    orwE = orwp.tile([128, 512], F32, name="orwE", tag="orw")
                    orwO = orwp.tile([128, 512], F32, name="orwO", tag="orw")
                    for kc in range(KCP):
                        ks = slice(kc * 128, kc * 128 + 128)
                        lg = lgp.tile([128, 1024], F32, name="lg", tag="lg")
                        mm(lg[:, 0:512], r(kT[0:64, c, ks]), r(qT[0:64, c, qs]),
                           start=True, stop=True)
                        mm(lg[:, 512:1024], r(kT[64:128, c, ks]), r(qT[64:128, c, qs]),
                           start=True, stop=True)
                        eeo = eeop.tile([128, 1024], BF16, name="eeo", tag="eeo")
                        nc.scalar.activation(eeo, lg, AF.Exp, scale=SCALE)
                        mm(orwE[0:65, :], ve_list[kc][:, he, :], eeo[:, 0:512],
                           start=(kc == 0), stop=(kc == KCP - 1))
                        mm(orwO[0:65, :], ve_list[kc][:, ho, :], eeo[:, 512:1024],
                           start=(kc == 0), stop=(kc == KCP - 1))
                    stE = stp.tile([1, 512], F32, name="stE", tag="st")
                    stO = stp.tile([1, 512], F32, name="stO", tag="st")
                    nc.vector.tensor_copy(r(stE), orwE[64:65, :])
                    nc.vector.tensor_copy(r(stO), orwO[64:65, :])
                    bps = psum()
                    mm(bps, r(selE), r(stE), start=True, stop=False)
                    mm(bps, r(selO), r(stO), start=False, stop=True)
                    bc = bcp.tile([128, 512], F32, name="bc", tag="bc")
                    nc.vector.reciprocal(bc, bps)
                    nc.vector.tensor_mul(orwE[0:64, :], orwE[0:64, :], bc[0:64, :])
                    nc.vector.tensor_mul(orwO[0:64, :], orwO[0:64, :], bc[64:128, :])
                    nc.vector.tensor_add(r(qT[0:64, c, qs]), qT[0:64, c, qs],
                                         orwE[0:64, :])
                    nc.vector.tensor_add(r(qT[64:128, c, qs]), qT[64:128, c, qs],
                                         orwO[0:64, :])

        for t in range(n_tasks):
            validc = maskp.tile([128, KCP], F32, name="validc", tag="validc")
            nc.sync.dma_start(out=validc,
                              in_=VAL_d[t].rearrange("(k p) -> p k", p=128))
            validc8 = maskp.tile([128, KCP, NH], BF16, name="validc8", tag="validc8")
            for rc in range(KCP):
                nc.vector.tensor_scalar(out=validc8[:, rc, :], in0=ones8,
                                        scalar1=validc[:, rc:rc + 1], scalar2=None,
                                        op0=AluOpType.mult)

            czT = czp.tile([128, ncp], F32, name="czT", tag="czT")
            nc.sync.dma_start(out=r(czT), in_=CT_d[t].bitcast(MM_DT))
            xcT = xp.tile([64, ncp], F32, name="xcT", tag="xT")
            nc.sync.dma_start(out=r(xcT), in_=XCT_d[t].bitcast(MM_DT))
            xdT = xp.tile([64, ND], F32, name="xdT", tag="xT")
            nc.sync.dma_start(out=r(xdT), in_=XDT_d[t].bitcast(MM_DT))

            q0T = qp.tile([128, CH, ND], F32, name="q0T", tag="q")
            proj64(Wq0s, xdT, q0T, QS)

            v0e = [None] * KCP
            v1e = [None] * KCP
            for mi, (ms0, ms1) in enumerate(MS):
                ms = slice(ms0, ms1)
                w = ms1 - ms0
                h1 = hp.tile([128, CH, 512], BF16, name="h1", tag="h")
                for c in range(CH):
                    ps = psum()
                    mm(ps[:, 0:w], r(W0s[:, c * 128:(c + 1) * 128]), r(czT[:, ms]),
                       start=True, stop=True)
                    nc.vector.tensor_scalar(out=h1[:, c, 0:w], in0=ps[:, 0:w],
                                            scalar1=b0c[:, c:c + 1], scalar2=0.0,
                                            op0=AluOpType.add, op1=AluOpType.max)
                h2 = hp.tile([128, CH, 512], BF16, name="h2", tag="h")
                for c in range(CH):
                    ps = psum()
                    for kc in range(CH):
                        mm(ps[:, 0:w], Wm0s[:, kc, c * 128:(c + 1) * 128],
                           h1[:, kc, 0:w], start=(kc == 0), stop=(kc == CH - 1))
                    nc.vector.tensor_scalar(out=h2[:, c, 0:w], in0=ps[:, 0:w],
                                            scalar1=bm0c[:, c:c + 1], scalar2=0.0,
                                            op0=AluOpType.add, op1=AluOpType.max)
                h3 = hp.tile([128, CH, 512], BF16, name="h3", tag="h")
                for c in range(CH):
                    ps = psum()
                    for kc in range(CH):
                        mm(ps[:, 0:w], Wm1s[:, kc, c * 128:(c + 1) * 128],
                           h2[:, kc, 0:w], start=(kc == 0), stop=(kc == CH - 1))
                    nc.vector.tensor_scalar(out=h3[:, c, 0:w], in0=ps[:, 0:w],
                                            scalar1=bm1c[:, c:c + 1], scalar2=0.0,
                                            op0=AluOpType.add, op1=AluOpType.max)
                d_t = dp.tile([128, CH, 512], BF16, name="d_t", tag="d")
                for c in range(CH):
                    ps = psum()
                    for kc in range(CH):
                        mm(ps[:, 0:w], Wls[:, kc, c * 128:(c + 1) * 128],
                           h3[:, kc, 0:w], start=(kc == 0), stop=(kc == CH - 1))
                    nc.vector.tensor_scalar(out=d_t[:, c, 0:w], in0=ps[:, 0:w],
                                            scalar1=dbc[:, c:c + 1], scalar2=None,
                                            op0=AluOpType.add)
                for j in range((ms1 - ms0) // 128):
                    rc = ms0 // 128 + j
                    js = slice(j * 128, j * 128 + 128)
                    for vlist, Wvs in ((v0e, Wv0s), (v1e, Wv1s)):
                        ps = psum()
                        for kc in range(CH):
                            mm(ps, d_t[:, kc, js], Wvs[:, kc, :],
                               start=(kc == 0), stop=(kc == CH - 1))
                        ve = vep.tile([128, NH, HD + 1], BF16, name="ve", tag="ve")
                        nc.vector.tensor_scalar(
                            out=ve[:, :, 0:HD],
                            in0=ps.rearrange("p (h e) -> p h e", h=NH),
                            scalar1=validc[:, rc:rc + 1], scalar2=None,
                            op0=AluOpType.mult)
                        nc.vector.tensor_copy(
                            ve[:, :, HD:HD + 1],
                            validc8[:, rc, :].rearrange("p (h a) -> p h a", a=1))
                        vlist[rc] = ve

            k0T = kp.tile([128, CH, ncp], F32, name="k0T", tag="k")
            proj64(Wk0s, xcT, k0T, MS)
            attn_block(q0T, k0T, v0e)

            u2T = qp.tile([128, CH, ND], F32, name="u2T", tag="q")
            for c in range(CH):
                for (qs0, qs1) in QS:
                    ms = slice(qs0, qs1)
                    ps = psum()
                    for kc in range(CH):
                        mm(ps, Wo0s[:, kc, c * 128:(c + 1) * 128].bitcast(MM_DT),
                           r(q0T[:, kc, ms]), start=(kc == 0), stop=(kc == CH - 1))
                    ft = ftp.tile([128, 512], F32, name="ft", tag="ft")
                    nc.vector.tensor_scalar(out=ft, in0=ps,
                                            scalar1=bo0c[:, c:c + 1], scalar2=0.0,
                                            op0=AluOpType.add, op1=AluOpType.max)
                    nc.vector.tensor_add(r(u2T[:, c, ms]), q0T[:, c, ms], ft)

            q1T = qp.tile([128, CH, ND], F32, name="q1T", tag="q")
            for c in range(CH):
                for (qs0, qs1) in QS:
                    ms = slice(qs0, qs1)
                    ps = psum()
                    for kc in range(CH):
                        mm(ps, Wq1s[:, kc, c * 128:(c + 1) * 128].bitcast(MM_DT),
                           r(u2T[:, kc, ms]), start=(kc == 0), stop=(kc == CH - 1))
                    nc.vector.tensor_copy(r(q1T[:, c, ms]), ps)

            k1T = kp.tile([128, CH, ncp], F32, name="k1T", tag="k")
            proj64(Wk1s, xcT, k1T, MS)
            attn_block(q1T, k1T, v1e)

            for j in range(ND // 128):
                js = slice(j * 128, j * 128 + 128)
                fp = psum()
                mm(fp, r(ones1), r(bo1r), start=True, stop=False)
                for kc in range(CH):
                    mm(fp, r(q1T[:, kc, js]), Wo1s[:, kc, :].bitcast(MM_DT),
                       start=False, stop=(kc == CH - 1))
                fr = ftp.tile([128, 512], F32, name="fr", tag="ft")
                nc.vector.tensor_scalar(out=fr, in0=fp, scalar1=0.0, scalar2=None,
                                        op0=AluOpType.max)
                ur = urp.tile([128, DH], F32, name="ur", tag="ur")
                for c in range(CH):
                    cs = slice(c * 128, c * 128 + 128)
                    tp = psum()
                    nc.tensor.transpose(tp[:, 0:128], q1T[:, c, js], ident)
                    nc.vector.tensor_add(ur[:, cs], fr[:, cs], tp[:, 0:128])
                nc.sync.dma_start(out=OUT_d[t, js, :], in_=ur)

    nc.compile()
    return nc


_NC_CACHE = {}


def _get_nc(ncp):
    key = (TPC, ncp)
    if key not in _NC_CACHE:
        _NC_CACHE[key] = build_nc(TPC, ncp)
    return _NC_CACHE[key]


def _as_f32(x):
    return np.ascontiguousarray(np.asarray(x, dtype=np.float32))


def _pack(C, XC):
    mask = np.isnan(C[:, :, -1])
    counts = (~mask).sum(1)
    ncp = int(min(NC, -(-int(counts.max()) // 128) * 128))
    order = np.argsort(mask, axis=1, kind="stable")[:, :ncp]
    Cc = np.take_along_axis(C, order[:, :, None], axis=1)
    XCc = np.take_along_axis(XC, order[:, :, None], axis=1)
    tail = np.arange(ncp)[None, :] >= counts[:, None]
    Cc[tail] = 0.0
    XCc[tail] = 0.0
    valid = np.ascontiguousarray((~tail).astype(np.float32))
    return np.ascontiguousarray(Cc), np.ascontiguousarray(XCc), valid, ncp


def run(inputs, trace=False, **kw):
    C = _as_f32(inputs["C"])
    XC = _as_f32(inputs["X_C"])
    XD = _as_f32(inputs["X_D"])
    Cc, XCc, valid, ncp = _pack(C, XC)
    CT = np.ascontiguousarray(Cc.transpose(0, 2, 1))
    XCT = np.ascontiguousarray(XCc.transpose(0, 2, 1))
    XDT = np.ascontiguousarray(XD.transpose(0, 2, 1))
    nc = _get_nc(ncp)

    bf = ml_dtypes.bfloat16
    weights = {
        "W0": _as_f32(inputs["W0"]), "b0": _as_f32(inputs["b0"]),
        "WmB": _as_f32(inputs["Wm"]).astype(bf), "bm": _as_f32(inputs["bm"]),
        "WlB": _as_f32(inputs["Wl"]).astype(bf), "bl": _as_f32(inputs["bl"]),
        "task_emb": _as_f32(inputs["task_emb"]),
        "Wq0": _as_f32(inputs["Wq0"]), "Wk0": _as_f32(inputs["Wk0"]),
        "Wv0B": _as_f32(inputs["Wv0"]).astype(bf),
        "Wo0": _as_f32(inputs["Wo0"]), "bo0": _as_f32(inputs["bo0"]),
        "Wq1": _as_f32(inputs["Wq1"]), "Wk1": _as_f32(inputs["Wk1"]),
        "Wv1B": _as_f32(inputs["Wv1"]).astype(bf),
        "Wo1": _as_f32(inputs["Wo1"]), "bo1": _as_f32(inputs["bo1"]),
    }
    in_maps = []
    for i in range(NCORES):
        s = slice(i * TPC, (i + 1) * TPC)
        m = dict(weights)
        m["CT"] = CT[s]
        m["XCT"] = XCT[s]
        m["XDT"] = XDT[s]
        m["VALID"] = valid[s]
        in_maps.append(m)
    res = run_bass_kernel_spmd(nc, in_maps, core_ids=list(range(NCORES)),
                               trace=trace, **kw)
    out = np.concatenate([res.results[i]["OUT"] for i in range(NCORES)], axis=0)
    return out, res


def kernel(**inputs) -> np.ndarray:
    out, _ = run(inputs, trace=False)
    return out


# revision 13
# speedup vs baseline: 1.2227x; 1.2227x over previous
import math
from contextlib import ExitStack

import ml_dtypes
import numpy as np

import concourse.bacc as bacc
import concourse.bass as bass
import concourse.tile as tile
from concourse import mybir
from concourse.alu_op_type import AluOpType
from concourse.bass_utils import run_bass_kernel_spmd
from concourse.masks import make_identity

B, NC, ND = 64, 1024, 1024
DX, DY, DH = 64, 64, 512
NH, HD = 8, 64
CH = 4
NCORES = 8
TPC = B // NCORES

F32 = mybir.dt.float32
F32R = mybir.dt.float32r
BF16 = mybir.dt.bfloat16
AF = mybir.ActivationFunctionType

SCALE = 1.0 / math.sqrt(DH)
MM_DT = F32R


def _blocks(n, step=512):
    return [(s, min(s + step, n)) for s in range(0, n, step)]


def build_nc(n_tasks, ncp):
    KCP = ncp // 128
    MS = _blocks(ncp)
    QS = _blocks(ND)

    nc = bacc.Bacc(None, target_bir_lowering=False, dynamic_dma_scratch_size=256)

    def r(ap):
        return ap.bitcast(MM_DT)

    def mm(out, lhsT, rhs, **kw):
        nc.tensor.matmul(out, lhsT, rhs, **kw)

    CT_d = nc.dram_tensor("CT", [n_tasks, DX + DY, ncp], F32, kind="ExternalInput")
    XCT_d = nc.dram_tensor("XCT", [n_tasks, DX, ncp], F32, kind="ExternalInput")
    XDT_d = nc.dram_tensor("XDT", [n_tasks, DX, ND], F32, kind="ExternalInput")
    VAL_d = nc.dram_tensor("VALID", [n_tasks, ncp], F32, kind="ExternalInput")
    W0_d = nc.dram_tensor("W0", [DX + DY, DH], F32, kind="ExternalInput")
    b0_d = nc.dram_tensor("b0", [DH], F32, kind="ExternalInput")
    WmB_d = nc.dram_tensor("WmB", [2, DH, DH], BF16, kind="ExternalInput")
    bm_d = nc.dram_tensor("bm", [2, DH], F32, kind="ExternalInput")
    WlB_d = nc.dram_tensor("WlB", [DH, DH], BF16, kind="ExternalInput")
    bl_d = nc.dram_tensor("bl", [DH], F32, kind="ExternalInput")
    temb_d = nc.dram_tensor("task_emb", [DH], F32, kind="ExternalInput")
    Wq0_d = nc.dram_tensor("Wq0", [DX, DH], F32, kind="ExternalInput")
    Wk0_d = nc.dram_tensor("Wk0", [DX, DH], F32, kind="ExternalInput")
    Wv0B_d = nc.dram_tensor("Wv0B", [DH, DH], BF16, kind="ExternalInput")
    Wo0_d = nc.dram_tensor("Wo0", [DH, DH], F32, kind="ExternalInput")
    bo0_d = nc.dram_tensor("bo0", [DH], F32, kind="ExternalInput")
    Wq1_d = nc.dram_tensor("Wq1", [DH, DH], F32, kind="ExternalInput")
    Wk1_d = nc.dram_tensor("Wk1", [DX, DH], F32, kind="ExternalInput")
    Wv1B_d = nc.dram_tensor("Wv1B", [DH, DH], BF16, kind="ExternalInput")
    Wo1_d = nc.dram_tensor("Wo1", [DH, DH], F32, kind="ExternalInput")
    bo1_d = nc.dram_tensor("bo1", [DH], F32, kind="ExternalInput")
    OUT_d = nc.dram_tensor("OUT", [n_tasks, ND, DH], F32, kind="ExternalOutput")

    with tile.TileContext(nc) as tc, ExitStack() as ctx, \
            nc.allow_low_precision(reason="fp32r/bf16 rounding is intentional"):
        wp = ctx.enter_context(tc.tile_pool(name="wp", bufs=1))
        czp = ctx.enter_context(tc.tile_pool(name="czp", bufs=2))
        xp = ctx.enter_context(tc.tile_pool(name="xp", bufs=3))
        maskp = ctx.enter_context(tc.tile_pool(name="maskp", bufs=2))
        hp = ctx.enter_context(tc.tile_pool(name="hp", bufs=2))
        dp = ctx.enter_context(tc.tile_pool(name="dp", bufs=2))
        vep = ctx.enter_context(tc.tile_pool(name="vep", bufs=18))
        qp = ctx.enter_context(tc.tile_pool(name="qp", bufs=3))
        kp = ctx.enter_context(tc.tile_pool(name="kp", bufs=2))
        eeop = ctx.enter_context(tc.tile_pool(name="eeop", bufs=4))
        ftp = ctx.enter_context(tc.tile_pool(name="ftp", bufs=2))
        bcp = ctx.enter_context(tc.tile_pool(name="bcp", bufs=2))
        stp = ctx.enter_context(tc.tile_pool(name="stp", bufs=4))
        urp = ctx.enter_context(tc.tile_pool(name="urp", bufs=2))
        accp = ctx.enter_context(tc.tile_pool(name="accp", bufs=2, space="PSUM"))
        orwp = ctx.enter_context(tc.tile_pool(name="orwp", bufs=2, space="PSUM"))
        lgp = ctx.enter_context(tc.tile_pool(name="lgp", bufs=2, space="PSUM"))

        def psum():
            return accp.tile([128, 512], F32, name="ps", tag="ps")

        ident = wp.tile([128, 128], F32, name="ident")
        make_identity(nc, ident)
        ones1f = wp.tile([1, 128], F32, name="ones1f")
        nc.vector.memset(ones1f, 1.0)
        ones1 = wp.tile([1, 128], F32, name="ones1")
        nc.vector.tensor_copy(r(ones1), ones1f)
        ones8 = wp.tile([128, NH], BF16, name="ones8")
        nc.vector.memset(ones8, 1.0)
        d2 = wp.tile([1, 128], mybir.dt.int32, name="d2")
        nc.gpsimd.iota(d2, pattern=[[1, 2], [0, 64]], channel_multiplier=0)
        d2f = wp.tile([1, 128], F32, name="d2f")
        nc.vector.tensor_copy(d2f, d2)
        selE = wp.tile([1, 128], F32, name="selE")
        nc.vector.tensor_scalar(out=r(selE), in0=d2f, scalar1=0.0, scalar2=None,
                                op0=AluOpType.is_equal)
        selO = wp.tile([1, 128], F32, name="selO")
        nc.vector.tensor_scalar(out=r(selO), in0=d2f, scalar1=1.0, scalar2=None,
                                op0=AluOpType.is_equal)

        def load_w_f32r(name, src_ap):
            t = wp.tile([128, CH, DH], F32, name=name, tag=name)
            nc.sync.dma_start(
                out=r(t), in_=src_ap.rearrange("(c p) n -> p c n", p=128).bitcast(MM_DT))
            return t

        def load_w_bf16(name, src_ap):
            t = wp.tile([128, CH, DH], BF16, name=name, tag=name)
            nc.sync.dma_start(out=t, in_=src_ap.rearrange("(c p) n -> p c n", p=128))
            return t

        def load_col(name, src_ap):
            t = wp.tile([128, CH], F32, name=name, tag=name)
            nc.sync.dma_start(out=t, in_=src_ap.rearrange("(c p) -> p c", p=128))
            return t

        W0s = wp.tile([128, DH], F32, name="W0s")
        nc.sync.dma_start(out=r(W0s), in_=W0_d[:, :].bitcast(MM_DT))
        Wm0s = load_w_bf16("Wm0s", WmB_d[0])
        Wm1s = load_w_bf16("Wm1s", WmB_d[1])
        Wls = load_w_bf16("Wls", WlB_d[:, :])
        Wv0s = load_w_bf16("Wv0s", Wv0B_d[:, :])
        Wv1s = load_w_bf16("Wv1s", Wv1B_d[:, :])
        Wo0s = load_w_f32r("Wo0s", Wo0_d[:, :])
        Wq1s = load_w_f32r("Wq1s", Wq1_d[:, :])
        Wo1s = load_w_f32r("Wo1s", Wo1_d[:, :])

        Wq0s = wp.tile([64, DH], F32, name="Wq0s")
        nc.sync.dma_start(out=r(Wq0s), in_=Wq0_d[:, :].bitcast(MM_DT))
        Wk0s = wp.tile([64, DH], F32, name="Wk0s")
        nc.sync.dma_start(out=r(Wk0s), in_=Wk0_d[:, :].bitcast(MM_DT))
        Wk1s = wp.tile([64, DH], F32, name="Wk1s")
        nc.sync.dma_start(out=r(Wk1s), in_=Wk1_d[:, :].bitcast(MM_DT))

        b0c = load_col("b0c", b0_d[:])
        bm0c = load_col("bm0c", bm_d[0])
        bm1c = load_col("bm1c", bm_d[1])
        blc = load_col("blc", bl_d[:])
        tembc = load_col("tembc", temb_d[:])
        bo0c = load_col("bo0c", bo0_d[:])
        dbc = wp.tile([128, CH], F32, name="dbc")
        nc.vector.tensor_add(dbc, blc, tembc)
        bo1r = wp.tile([1, DH], F32, name="bo1r")
        nc.sync.dma_start(out=r(bo1r),
                          in_=bo1_d[:].rearrange("(a n) -> a n", a=1).bitcast(MM_DT))

        def proj64(Ws, xT, outT, width_blocks):
            for c in range(CH):
                for (s, e) in width_blocks:
                    w = e - s
                    ps = psum()
                    mm(ps[:, 0:w], r(Ws[:, c * 128:(c + 1) * 128]), r(xT[:, s:e]),
                       start=True, stop=True)
                    nc.vector.tensor_copy(r(outT[:, c, s:e]), ps[:, 0:w])

        def attn_block(qT, kT, ve_list):
            for hp2 in range(NH // 2):
                he, ho, c = 2 * hp2, 2 * hp2 + 1, hp2
                for (qs0, qs1) in QS:
                    qs = slice(qs0, qs1)
                    orwE = orwp.tile([128, 512], F32, name="orwE", tag="orw")
                    orwO = orwp.tile([128, 512], F32, name="orwO", tag="orw")
                    for kc in range(KCP):
                        ks = slice(kc * 128, kc * 128 + 128)
                        lg = lgp.tile([128, 1024], F32, name="lg", tag="lg")
                        mm(lg[:, 0:512], r(kT[0:64, c, ks]), r(qT[0:64, c, qs]),
                           start=True, stop=True)
                        mm(lg[:, 512:1024], r(kT[64:128, c, ks]), r(qT[64:128, c, qs]),
                           start=True, stop=True)
                        eeo = eeop.tile([128, 1024], BF16, name="eeo", tag="eeo")
                        nc.scalar.activation(eeo, lg, AF.Exp, scale=SCALE)
                        mm(orwE[0:65, :], ve_list[kc][:, he, :], eeo[:, 0:512],
                           start=(kc == 0), stop=(kc == KCP - 1))
                        mm(orwO[0:65, :], ve_list[kc][:, ho, :], eeo[:, 512:1024],
                           start=(kc == 0), stop=(kc == KCP - 1))
                    stE = stp.tile([1, 512], F32, name="stE", tag="st")
                    stO = stp.tile([1, 512], F32, name="stO", tag="st")
                    nc.vector.tensor_copy(r(stE), orwE[64:65, :])
                    nc.vector.tensor_copy(r(stO), orwO[64:65, :])
                    bps = psum()
                    mm(bps, r(selE), r(stE), start=True, stop=False)
                    mm(bps, r(selO), r(stO), start=False, stop=True)
                    bc = bcp.tile([128, 512], F32, name="bc", tag="bc")
                    nc.vector.reciprocal_approx_fast(out=bc, in_=bps)
                    nc.vector.tensor_mul(orwE[0:64, :], orwE[0:64, :], bc[0:64, :])
                    nc.vector.tensor_mul(orwO[0:64, :], orwO[0:64, :], bc[64:128, :])
                    nc.vector.tensor_add(r(qT[0:64, c, qs]), qT[0:64, c, qs],
                                         orwE[0:64, :])
                    nc.vector.tensor_add(r(qT[64:128, c, qs]), qT[64:128, c, qs],
                                         orwO[0:64, :])

        for t in range(n_tasks):
            validc = maskp.tile([128, KCP], F32, name="validc", tag="validc")
            nc.sync.dma_start(out=validc,
                              in_=VAL_d[t].rearrange("(k p) -> p k", p=128))
            validc8 = maskp.tile([128, KCP, NH], BF16, name="validc8", tag="validc8")
            for rc in range(KCP):
                nc.vector.tensor_scalar(out=validc8[:, rc, :], in0=ones8,
                                        scalar1=validc[:, rc:rc + 1], scalar2=None,
                                        op0=AluOpType.mult)

            czT = czp.tile([128, ncp], F32, name="czT", tag="czT")
            nc.sync.dma_start(out=r(czT), in_=CT_d[t].bitcast(MM_DT))
            xcT = xp.tile([64, ncp], F32, name="xcT", tag="xT")
            nc.sync.dma_start(out=r(xcT), in_=XCT_d[t].bitcast(MM_DT))
            xdT = xp.tile([64, ND], F32, name="xdT", tag="xT")
            nc.sync.dma_start(out=r(xdT), in_=XDT_d[t].bitcast(MM_DT))

            q0T = qp.tile([128, CH, ND], F32, name="q0T", tag="q")
            proj64(Wq0s, xdT, q0T, QS)

            v0e = [None] * KCP
            v1e = [None] * KCP
            for mi, (ms0, ms1) in enumerate(MS):
                ms = slice(ms0, ms1)
                w = ms1 - ms0
                h1 = hp.tile([128, CH, 512], BF16, name="h1", tag="h")
                for c in range(CH):
                    ps = psum()
                    mm(ps[:, 0:w], r(W0s[:, c * 128:(c + 1) * 128]), r(czT[:, ms]),
                       start=True, stop=True)
                    nc.vector.tensor_scalar(out=h1[:, c, 0:w], in0=ps[:, 0:w],
                                            scalar1=b0c[:, c:c + 1], scalar2=0.0,
                                            op0=AluOpType.add, op1=AluOpType.max)
                h2 = hp.tile([128, CH, 512], BF16, name="h2", tag="h")
                for c in range(CH):
                    ps = psum()
                    for kc in range(CH):
                        mm(ps[:, 0:w], Wm0s[:, kc, c * 128:(c + 1) * 128],
                           h1[:, kc, 0:w], start=(kc == 0), stop=(kc == CH - 1))
                    nc.vector.tensor_scalar(out=h2[:, c, 0:w], in0=ps[:, 0:w],
                                            scalar1=bm0c[:, c:c + 1], scalar2=0.0,
                                            op0=AluOpType.add, op1=AluOpType.max)
                h3 = hp.tile([128, CH, 512], BF16, name="h3", tag="h")
                for c in range(CH):
                    ps = psum()
                    for kc in range(CH):
                        mm(ps[:, 0:w], Wm1s[:, kc, c * 128:(c + 1) * 128],
                           h2[:, kc, 0:w], start=(kc == 0), stop=(kc == CH - 1))
                    nc.vector.tensor_scalar(out=h3[:, c, 0:w], in0=ps[:, 0:w],
                                            scalar1=bm1c[:, c:c + 1], scalar2=0.0,
                                            op0=AluOpType.add, op1=AluOpType.max)
                d_t = dp.tile([128, CH, 512], BF16, name="d_t", tag="d")
                for c in range(CH):
                    ps = psum()
                    for kc in range(CH):
                        mm(ps[:, 0:w], Wls[:, kc, c * 128:(c + 1) * 128],
                           h3[:, kc, 0:w], start=(kc == 0), stop=(kc == CH - 1))
                    nc.vector.tensor_scalar(out=d_t[:, c, 0:w], in0=ps[:, 0:w],
                                            scalar1=dbc[:, c:c + 1], scalar2=None,
                                            op0=AluOpType.add)
                for j in range((ms1 - ms0) // 128):
                    rc = ms0 // 128 + j
                    js = slice(j * 128, j * 128 + 128)
                    for vlist, Wvs in ((v0e, Wv0s), (v1e, Wv1s)):
                        ps = psum()
                        for kc in range(CH):
                            mm(ps, d_t[:, kc, js], Wvs[:, kc, :],
                               start=(kc == 0), stop=(kc == CH - 1))
                        ve = vep.tile([128, NH, HD + 1], BF16, name="ve", tag="ve")
                        nc.vector.tensor_scalar(
                            out=ve[:, :, 0:HD],
                            in0=ps.rearrange("p (h e) -> p h e", h=NH),
                            scalar1=validc[:, rc:rc + 1], scalar2=None,
                            op0=AluOpType.mult)
                        nc.vector.tensor_copy(
                            ve[:, :, HD:HD + 1],
                            validc8[:, rc, :].rearrange("p (h a) -> p h a", a=1))
                        vlist[rc] = ve

            k0T = kp.tile([128, CH, ncp], F32, name="k0T", tag="k")
            proj64(Wk0s, xcT, k0T, MS)
            attn_block(q0T, k0T, v0e)

            u2T = qp.tile([128, CH, ND], F32, name="u2T", tag="q")
            for c in range(CH):
                for (qs0, qs1) in QS:
                    ms = slice(qs0, qs1)
                    ps = psum()
                    for kc in range(CH):
                        mm(ps, Wo0s[:, kc, c * 128:(c + 1) * 128].bitcast(MM_DT),
                           r(q0T[:, kc, ms]), start=(kc == 0), stop=(kc == CH - 1))
                    ft = ftp.tile([128, 512], F32, name="ft", tag="ft")
                    nc.vector.tensor_scalar(out=ft, in0=ps,
                                            scalar1=bo0c[:, c:c + 1], scalar2=0.0,
                                            op0=AluOpType.add, op1=AluOpType.max)
                    nc.vector.tensor_add(r(u2T[:, c, ms]), q0T[:, c, ms], ft)

            q1T = qp.tile([128, CH, ND], F32, name="q1T", tag="q")
            for c in range(CH):
                for (qs0, qs1) in QS:
                    ms = slice(qs0, qs1)
                    ps = psum()
                    for kc in range(CH):
                        mm(ps, Wq1s[:, kc, c * 128:(c + 1) * 128].bitcast(MM_DT),
                           r(u2T[:, kc, ms]), start=(kc == 0), stop=(kc == CH - 1))
                    nc.vector.tensor_copy(r(q1T[:, c, ms]), ps)

            k1T = kp.tile([128, CH, ncp], F32, name="k1T", tag="k")
            proj64(Wk1s, xcT, k1T, MS)
            attn_block(q1T, k1T, v1e)

            for j in range(ND // 128):
                js = slice(j * 128, j * 128 + 128)
                fp = psum()
                mm(fp, r(ones1), r(bo1r), start=True, stop=False)
                for kc in range(CH):
                    mm(fp, r(q1T[:, kc, js]), Wo1s[:, kc, :].bitcast(MM_DT),
                       start=False, stop=(kc == CH - 1))
                fr = ftp.tile([128, 512], F32, name="fr", tag="ft")
                nc.vector.tensor_scalar(out=fr, in0=fp, scalar1=0.0, scalar2=None,
                                        op0=AluOpType.max)
                ur = urp.tile([128, DH], F32, name="ur", tag="ur")
                for c in range(CH):
                    cs = slice(c * 128, c * 128 + 128)
                    tp = psum()
                    nc.tensor.transpose(tp[:, 0:128], q1T[:, c, js], ident)
                    nc.vector.tensor_add(ur[:, cs], fr[:, cs], tp[:, 0:128])
                nc.sync.dma_start(out=OUT_d[t, js, :], in_=ur)

    nc.compile()
    return nc


_NC_CACHE = {}


def _get_nc(ncp):
    key = (TPC, ncp)
    if key not in _NC_CACHE:
        _NC_CACHE[key] = build_nc(TPC, ncp)
    return _NC_CACHE[key]


def _as_f32(x):
    return np.ascontiguousarray(np.asarray(x, dtype=np.float32))


def _pack(C, XC):
    mask = np.isnan(C[:, :, -1])
    counts = (~mask).sum(1)
    ncp = int(min(NC, -(-int(counts.max()) // 128) * 128))
    order = np.argsort(mask, axis=1, kind="stable")[:, :ncp]
    Cc = np.take_along_axis(C, order[:, :, None], axis=1)
    XCc = np.take_along_axis(XC, order[:, :, None], axis=1)
    tail = np.arange(ncp)[None, :] >= counts[:, None]
    Cc[tail] = 0.0
    XCc[tail] = 0.0
    valid = np.ascontiguousarray((~tail).astype(np.float32))
    return np.ascontiguousarray(Cc), np.ascontiguousarray(XCc), valid, ncp


def run(inputs, trace=False, **kw):
    C = _as_f32(inputs["C"])
    XC = _as_f32(inputs["X_C"])
    XD = _as_f32(inputs["X_D"])
    Cc, XCc, valid, ncp = _pack(C, XC)
    CT = np.ascontiguousarray(Cc.transpose(0, 2, 1))
    XCT = np.ascontiguousarray(XCc.transpose(0, 2, 1))
    XDT = np.ascontiguousarray(XD.transpose(0, 2, 1))
    nc = _get_nc(ncp)

    bf = ml_dtypes.bfloat16
    weights = {
        "W0": _as_f32(inputs["W0"]), "b0": _as_f32(inputs["b0"]),
        "WmB": _as_f32(inputs["Wm"]).astype(bf), "bm": _as_f32(inputs["bm"]),
        "WlB": _as_f32(inputs["Wl"]).astype(bf), "bl": _as_f32(inputs["bl"]),
        "task_emb": _as_f32(inputs["task_emb"]),
        "Wq0": _as_f32(inputs["Wq0"]), "Wk0": _as_f32(inputs["Wk0"]),
        "Wv0B": _as_f32(inputs["Wv0"]).astype(bf),
        "Wo0": _as_f32(inputs["Wo0"]), "bo0": _as_f32(inputs["bo0"]),
        "Wq1": _as_f32(inputs["Wq1"]), "Wk1": _as_f32(inputs["Wk1"]),
        "Wv1B": _as_f32(inputs["Wv1"]).astype(bf),
        "Wo1": _as_f32(inputs["Wo1"]), "bo1": _as_f32(inputs["bo1"]),
    }
    in_maps = []
    for i in range(NCORES):
        s = slice(i * TPC, (i + 1) * TPC)
        m = dict(weights)
        m["CT"] = CT[s]
        m["XCT"] = XCT[s]
        m["XDT"] = XDT[s]
        m["VALID"] = valid[s]
        in_maps.append(m)
    res = run_bass_kernel_spmd(nc, in_maps, core_ids=list(range(NCORES)),
                               trace=trace, **kw)
    out = np.concatenate([res.results[i]["OUT"] for i in range(NCORES)], axis=0)
    return out, res


def kernel(**inputs) -> np.ndarray:
    out, _ = run(inputs, trace=False)
    return out
